# revision 17
# baseline (speedup 1.0000x reference)
"""DGM-net forward kernel for Trainium2, 8-core data parallel.

Network (per batch row x of width 101, n_nodes=512, 3 layers):
    S = tanh(x @ W0 + b0)
    for i in 0..2:
        Z = tanh(x @ Uz[i] + S @ Wz[i] + bz[i])
        G = tanh(x @ Ug[i] + S @ Wg[i] + bg[i])
        R = tanh(x @ Ur[i] + S @ Wr[i] + br[i])
        H = tanh(x @ Uh[i] + (S*R) @ Wh[i] + bh[i])
        S = (1-G)*H + Z*S
    out = S @ Wf + bf

Layout: activations feature-major ([feature partitions, batch free]) so
every matmul uses the weight in NATURAL layout as stationary lhsT and
the activation as moving rhs.  X is host-transposed and shipped as bf16
"XT" with a ones row at partition 0; every bias is folded into the
matmul (U/W0 stationaries carry the bias as row 0), so ACT instructions
are bias-free and can span two PSUM banks.

HW-microbenchmarked facts (mb.py, REAL random data -- the zero-data
regime is ~1.5x faster and misleading):
  bf16  MM [128x128]x[128x512]    ~199 ns   <- fastest real-data option
  fp32r MM                        ~220 ns
  ACT tanh pair [128,2x512]       ~690 ns
  DVE op [128,512]                ~370 ns
  cross-engine dependency edge    ~1.1 us   -- 10x the cost model
The whole compute path is bf16 (numpy-validated rel err 7e-3 vs 2e-2
budget; PSUM accumulation stays fp32): 10% faster matmul stream AND
half the SBUF/DMA.  The ~1.1us/edge handoff makes serial MM->ACT->DVE->
MM chains expensive, so every layer boundary is covered with independent
PE work: the previous chunk's deferred final, chunk c+2's S0, and the
next layer's hoisted X@U matmuls; within a group the S@W contraction is
kt-major so the first-half S update unlocks half the matmuls early.
"""
import numpy as np
import ml_dtypes
from contextlib import ExitStack

import concourse.bacc as bacc
import concourse.mybir as mybir
import concourse.tile as tile
from concourse.bass_utils import run_bass_kernel_spmd


N_CORES = 8
B_FULL = 65536
B = B_FULL // N_CORES      # rows per core
D = 101                    # input width
DA = D + 1                 # augmented with ones row (bias fold)
N = 512                    # n_nodes
L = 3                      # layers
BT = 512                   # batch chunk (free dim of matmuls)
NT = N // 128              # output-feature tiles per gate
KT = N // 128              # contraction tiles for S@W
NP = NT // 2               # two-bank pair groups per gate
FP = mybir.dt.float32
BF = mybir.dt.bfloat16

GATES = ("z", "g", "r", "h")


def _build(reps=1, ablate_dve=False):
    nc = bacc.Bacc(None)
    Tanh = mybir.ActivationFunctionType.Tanh

    XTd = nc.declare_dram_parameter("XT", [DA, B], BF, isOutput=False)
    W0d = nc.declare_dram_parameter("W0a", [DA, N], BF, isOutput=False)
    Ud = {g: nc.declare_dram_parameter(f"U{g}a", [L, DA, N], BF, isOutput=False)
          for g in GATES}
    Wd = {g: nc.declare_dram_parameter(f"W{g}", [L, N, N], BF, isOutput=False)
          for g in GATES}
    Wfd = nc.declare_dram_parameter("Wf", [N, 1], BF, isOutput=False)
    bfd = nc.declare_dram_parameter("bfc", [1, 1], BF, isOutput=False)
    OUT = nc.declare_dram_parameter("out", [B, 1], FP, isOutput=True)

    with tile.TileContext(nc) as tc, ExitStack() as ctx:
        consts = ctx.enter_context(tc.tile_pool(name="consts", bufs=1))
        xtpool = ctx.enter_context(tc.tile_pool(name="xt", bufs=4))
        spool = ctx.enter_context(tc.tile_pool(name="s", bufs=4))
        zpool = ctx.enter_context(tc.tile_pool(name="z", bufs=2))
        gpool = ctx.enter_context(tc.tile_pool(name="g", bufs=2))
        rpool = ctx.enter_context(tc.tile_pool(name="r", bufs=2))
        hpool = ctx.enter_context(tc.tile_pool(name="h", bufs=2))
        opool = ctx.enter_context(tc.tile_pool(name="o", bufs=2))
        # pair-granular PSUM: each tile spans TWO banks ([128, 2, 512] fp32)
        psum = ctx.enter_context(tc.tile_pool(name="psum", bufs=3, space="PSUM"))
        psum_f = ctx.enter_context(tc.tile_pool(name="psum_f", bufs=2, space="PSUM"))

        # --- resident weights, natural (k-major) layout, via SWDGE ---
        def wdma(out, in_):
            nc.gpsimd.dma_start(out=out, in_=in_)

        w0 = consts.tile([DA, N], BF)
        bfc = consts.tile([1, 1], BF)
        u0, w0g, u12, w12 = {}, {}, {}, {}
        for g in GATES:
            u0[g] = consts.tile([DA, N], BF, name=f"u0_{g}")
            w0g[g] = consts.tile([128, KT, N], BF, name=f"w0_{g}")
            u12[g] = consts.tile([DA, L - 1, N], BF, name=f"u12_{g}")
            w12[g] = consts.tile([128, L - 1, KT, N], BF, name=f"w12_{g}")
        wf = consts.tile([128, KT], BF)

        def u_ap(g, l, c0, c1):
            return u0[g][:, c0:c1] if l == 0 else u12[g][:, l - 1, c0:c1]

        def w_ap(g, l, kt, c0, c1):
            return (w0g[g][:, kt, c0:c1] if l == 0
                    else w12[g][:, l - 1, kt, c0:c1])

        def emit_weight_dmas():
            nc.sync.dma_start(out=bfc[:], in_=bfd[:])
            wdma(w0[:], W0d[:])
            # first-consumed layer-0 weights first (gate order r,z,g,h)
            for g in ("r", "z", "g", "h"):
                wdma(u0[g][:], Ud[g][0].rearrange("p n -> p n"))
                wdma(w0g[g][:, 0:2],
                     Wd[g][0, 0:256].rearrange("(kt p) n -> p kt n", p=128))
                wdma(w0g[g][:, 2:4],
                     Wd[g][0, 256:512].rearrange("(kt p) n -> p kt n", p=128))
            for g in ("r", "z", "g", "h"):
                wdma(u12[g][:], Ud[g][1:3].rearrange("l p n -> p l n"))
                wdma(w12[g][:, 0],
                     Wd[g][1].rearrange("(kt p) n -> p kt n", p=128))
            for g in ("r", "z", "g", "h"):
                wdma(w12[g][:, 1],
                     Wd[g][2].rearrange("(kt p) n -> p kt n", p=128))
            wdma(wf[:], Wfd[:].rearrange("(kt p) o -> p (kt o)", p=128))

        sub, mult = mybir.AluOpType.subtract, mybir.AluOpType.mult

        def load_xt(c):
            xt = xtpool.tile([DA, BT], BF)
            if c == 0:
                h = BT // 2
                nc.sync.dma_start(out=xt[:, 0:h], in_=XTd[:, 0:h])
                nc.sync.dma_start(out=xt[:, h:BT], in_=XTd[:, h:BT])
            else:
                eng = nc.scalar if c == 1 else nc.sync
                eng.dma_start(out=xt[:], in_=XTd[:, c * BT:(c + 1) * BT])
            return xt

        def emit_s0_pair(xt, s, np_):
            # S0 = tanh(X_aug @ W0_aug), one two-bank pair group
            acc = psum.tile([128, 2, BT], FP, name="acc")
            for i in range(2):
                nt = 2 * np_ + i
                nc.tensor.matmul(acc[:, i, :], w0[:, nt * 128:(nt + 1) * 128],
                                 xt[:], start=True, stop=True)
            nc.scalar.activation(s[:, 2 * np_:2 * np_ + 2, :], acc[:], Tanh)

        def emit_xu(acc, g, l, xt, np_):
            # the two S-independent X@U matmuls of a pair group
            for i in range(2):
                nt = 2 * np_ + i
                nc.tensor.matmul(
                    acc[:, i, :], u_ap(g, l, nt * 128, (nt + 1) * 128),
                    xt[:], start=True, stop=False)

        def emit_sw(acc, g, l, src, np_, dest):
            # S@W contraction kt-MAJOR across both bank slices (first-half
            # S update unlocks 4 of 8 matmuls early), then the pair ACT
            for kt in range(KT):
                for i in range(2):
                    nt = 2 * np_ + i
                    nc.tensor.matmul(
                        acc[:, i, :],
                        w_ap(g, l, kt, nt * 128, (nt + 1) * 128),
                        src[:, kt, :], start=False, stop=(kt == KT - 1))
            nc.scalar.activation(dest[:, 2 * np_:2 * np_ + 2, :], acc[:], Tanh)

        def emit_gate_pair(g, l, xt, src, np_, dest):
            acc = psum.tile([128, 2, BT], FP, name="acc")
            emit_xu(acc, g, l, xt, np_)
            emit_sw(acc, g, l, src, np_, dest)

        def chunk_units(c, xt, s):
            """Generator: 31 work units (10 per layer + final).  Two of
            these run phase-offset-interleaved so every dependency stall
            of one chunk is covered by the other's matmul units."""
            for l in range(L):
                rt = rpool.tile([128, NT, BT], BF)
                zt = zpool.tile([128, NT, BT], BF)
                gt = gpool.tile([128, NT, BT], BF)
                ht = hpool.tile([128, NT, BT], BF)
                # R first: hides the R-ACT -> R-mul -> H chain under Z/G
                for np_ in range(NP):
                    emit_gate_pair("r", l, xt, s, np_, rt)
                    yield
                for np_ in range(NP):
                    emit_gate_pair("z", l, xt, s, np_, zt)
                    yield
                # DVE 1: R <- S*R (feeds H); Z <- Z*S (in place, reads the
                # OLD S before the layer-end sub overwrites it)
                if not ablate_dve:
                    for hf in range(2):
                        cs = slice(2 * hf, 2 * hf + 2)
                        nc.vector.tensor_mul(rt[:, cs, :], s[:, cs, :],
                                             rt[:, cs, :])
                    for hf in range(2):
                        cs = slice(2 * hf, 2 * hf + 2)
                        nc.vector.tensor_mul(zt[:, cs, :], zt[:, cs, :],
                                             s[:, cs, :])
                yield
                for np_ in range(NP):
                    emit_gate_pair("g", l, xt, s, np_, gt)
                    yield
                for np_ in range(NP):
                    emit_gate_pair("h", l, xt, s if ablate_dve else rt,
                                   np_, ht)
                    yield
                # DVE 2: S = (Z*S) - (G-1)*H, half-gate granular
                if not ablate_dve:
                    for hf in range(2):
                        cs = slice(2 * hf, 2 * hf + 2)
                        nc.vector.scalar_tensor_tensor(
                            gt[:, cs, :], gt[:, cs, :], 1.0, ht[:, cs, :],
                            op0=sub, op1=mult)          # (G-1)*H
                        nc.vector.tensor_sub(s[:, cs, :], zt[:, cs, :],
                                             gt[:, cs, :])
                yield
            emit_final(c, s, xt)
            yield

        def emit_final(c, s, xt_live):
            # out = S @ Wf + bf (bf lands via a K=1 matmul on the ones row)
            accf = psum_f.tile([1, BT], FP)
            nc.tensor.matmul(accf[:], bfc[:], xt_live[0:1, :],
                             start=True, stop=False)
            for kt in range(KT):
                nc.tensor.matmul(accf[:], wf[:, kt:kt + 1], s[:, kt, :],
                                 start=False, stop=(kt == KT - 1))
            ot = opool.tile([1, BT], FP)
            nc.scalar.activation(ot[:], accf[:],
                                 mybir.ActivationFunctionType.Copy)
            r0 = c * BT
            nc.sync.dma_start(out=OUT[r0:r0 + BT, 0:1].rearrange("b o -> o b"),
                              in_=ot[:])

        # slot pattern per 20-slot period: strict alternation EXCEPT a
        # triple-pull of the partner right after each chunk's layer-end
        # DVE unit, so the dependent next-layer matmuls always have ~4-6us
        # of foreign PE work queued ahead of them in the PE FIFO (the PE
        # sets aside at most 4 unsatisfied instructions before hard-
        # stalling, so the cover must PRECEDE the dependent matmuls).
        PATTERN = "ABABABABABBBABABABAA"
        SENTINEL = object()

        def pull(g):
            return next(g, SENTINEL) is not SENTINEL

        def emit_all():
            n_chunks = B // BT
            xts = {0: load_xt(0), 1: load_xt(1)}
            # startup S0 for the first pair (batch-halved chunk 0 so the
            # PE starts on the first xt half-transfer)
            s_a = spool.tile([128, KT, BT], BF, name="s")
            for h in range(2):
                c0, c1 = h * 256, (h + 1) * 256
                for np_ in range(NP):
                    acc = psum.tile([128, 2, BT], FP, name="acc")
                    for i in range(2):
                        nt = 2 * np_ + i
                        nc.tensor.matmul(acc[:, i, 0:256],
                                         w0[:, nt * 128:(nt + 1) * 128],
                                         xts[0][:, c0:c1], start=True,
                                         stop=True)
                    nc.scalar.activation(s_a[:, 2 * np_:2 * np_ + 2, c0:c1],
                                         acc[:, :, 0:256], Tanh)
            s_b = spool.tile([128, KT, BT], BF, name="s")
            for np_ in range(NP):
                emit_s0_pair(xts[1], s_b, np_)
            for p in range(n_chunks // 2):
                ca, cb = 2 * p, 2 * p + 1
                if ca + 2 < n_chunks:
                    xts[ca + 2] = load_xt(ca + 2)
                if cb + 2 < n_chunks:
                    xts[cb + 2] = load_xt(cb + 2)
                ga = chunk_units(ca, xts[ca], s_a)
                gb = chunk_units(cb, xts[cb], s_b)
                for _ in range(5):          # phase offset: A runs 5 ahead
                    pull(ga)
                a_alive = b_alive = True
                while a_alive:
                    for ch in PATTERN:
                        if ch == "A":
                            a_alive = pull(ga) and a_alive
                        else:
                            b_alive = pull(gb) and b_alive
                # drain B, interleaving the next pair's S0 pair-groups as
                # cover for B's tail chains (next-A's S0 first: the next
                # pair's head consumes s_a2 almost immediately)
                tails = []
                if ca + 2 < n_chunks:
                    s_a2 = spool.tile([128, KT, BT], BF, name="s")
                    s_b2 = spool.tile([128, KT, BT], BF, name="s")
                    tails = [lambda np_=np_: emit_s0_pair(xts[ca + 2], s_a2, np_)
                             for np_ in range(NP)]
                    tails += [lambda np_=np_: emit_s0_pair(xts[cb + 2], s_b2, np_)
                              for np_ in range(NP)]
                    s_a, s_b = s_a2, s_b2
                while b_alive:
                    b_alive = pull(gb)
                    if tails:
                        tails.pop(0)()
                while tails:
                    tails.pop(0)()

        emit_weight_dmas()
        if reps == 1:
            emit_all()
        else:           # device-side repetition loop, for benchmarking only
            with tc.For_i(0, reps):
                emit_all()

    nc.compile()
    return nc


_NC = None


def _get_nc():
    global _NC
    if _NC is None:
        _NC = _build()
    return _NC


def _bf(a):
    return np.ascontiguousarray(
        np.asarray(a, np.float32).astype(ml_dtypes.bfloat16))


def prep_shared(inputs):
    """bf16-convert weights; augment U-type weights with their bias as
    ROW 0 (matches the ones row at partition 0 of XT)."""
    shared = {}
    for g in GATES:
        shared[f"W{g}"] = _bf(inputs[f"W{g}"])
        U = np.asarray(inputs[f"U{g}"], np.float32)          # [L, D, N]
        b = np.asarray(inputs[f"b{g}"], np.float32)          # [L, 1, N]
        shared[f"U{g}a"] = _bf(
            np.concatenate([b.reshape(L, 1, N), U], axis=1))  # [L, DA, N]
    W0 = np.asarray(inputs["W0"], np.float32)                # [D, N]
    b0 = np.asarray(inputs["b0"], np.float32)                # [1, N]
    shared["W0a"] = _bf(np.concatenate([b0, W0], axis=0))
    shared["Wf"] = _bf(inputs["Wf"])
    shared["bfc"] = _bf(np.asarray(inputs["bf"], np.float32).reshape(1, 1))
    return shared


def prep_xt(Xcore):
    """[B, D] batch-major core shard -> [DA, B] bf16 feature-major,
    ones row 0."""
    Xt = np.asarray(Xcore, np.float32).T                     # [D, B]
    ones = np.ones((1, Xt.shape[1]), np.float32)
    return _bf(np.concatenate([ones, Xt], axis=0))


def _run(inputs, **kw):
    nc = _get_nc()
    shared = prep_shared(inputs)
    X = np.asarray(inputs["X"], np.float32)
    in_maps = [dict(shared, XT=prep_xt(X[i * B:(i + 1) * B]))
               for i in range(N_CORES)]
    res = run_bass_kernel_spmd(nc, in_maps, list(range(N_CORES)), **kw)
    out = np.concatenate([res.results[i]["out"] for i in range(N_CORES)], axis=0)
    return out, res


def kernel(**inputs) -> np.ndarray:
    out, _ = _run(inputs)
    return out


# revision 25
# speedup vs baseline: 1.0418x; 1.0418x over previous
"""DGM-net forward kernel for Trainium2, 8-core data parallel.

Network (per batch row x of width 101, n_nodes=512, 3 layers):
    S = tanh(x @ W0 + b0)
    for i in 0..2:
        Z = tanh(x @ Uz[i] + S @ Wz[i] + bz[i])
        G = tanh(x @ Ug[i] + S @ Wg[i] + bg[i])
        R = tanh(x @ Ur[i] + S @ Wr[i] + br[i])
        H = tanh(x @ Uh[i] + (S*R) @ Wh[i] + bh[i])
        S = (1-G)*H + Z*S
    out = S @ Wf + bf

Layout: activations feature-major ([feature partitions, batch free]) so
every matmul uses the weight in NATURAL layout as stationary lhsT and
the activation as moving rhs.  X is host-transposed and shipped as bf16
"XT" with a ones row at partition 0; every bias is folded into the
matmul (U/W0 stationaries carry the bias as row 0), so ACT instructions
are bias-free and can span two PSUM banks.

HW-microbenchmarked facts (mb.py, REAL random data -- the zero-data
regime is ~1.5x faster and misleading):
  bf16  MM [128x128]x[128x512]    ~199 ns   <- fastest real-data option
  fp32r MM                        ~220 ns
  ACT tanh pair [128,2x512]       ~690 ns
  DVE op [128,512]                ~370 ns
  cross-engine dependency edge    ~1.1 us   -- 10x the cost model
The whole compute path is bf16 (numpy-validated rel err 7e-3 vs 2e-2
budget; PSUM accumulation stays fp32): 10% faster matmul stream AND
half the SBUF/DMA.  The ~1.1us/edge handoff makes serial MM->ACT->DVE->
MM chains expensive, so FOUR chunks run round-robin interleaved one
work-unit at a time: every chunk's dependent next-layer matmuls enter
the PE FIFO ~3 foreign units (~6us of independent matmul work) after
their producer chain started.  Layer-boundary slots additionally carry
the S0 prefetch pair-groups of the next quad's chunks; within a group
the S@W contraction is kt-major so the first-half S update unlocks half
the matmuls early, and the two X@U matmuls (bias folded) go first.
"""
import numpy as np
import ml_dtypes
from contextlib import ExitStack

import concourse.bacc as bacc
import concourse.mybir as mybir
import concourse.tile as tile
from concourse.bass_utils import run_bass_kernel_spmd


N_CORES = 8
B_FULL = 65536
B = B_FULL // N_CORES      # rows per core
D = 101                    # input width
DA = D + 1                 # augmented with ones row (bias fold)
N = 512                    # n_nodes
L = 3                      # layers
BT = 512                   # batch chunk (free dim of matmuls)
NT = N // 128              # output-feature tiles per gate
KT = N // 128              # contraction tiles for S@W
NP = NT // 2               # two-bank pair groups per gate
FP = mybir.dt.float32
BF = mybir.dt.bfloat16

GATES = ("z", "g", "r", "h")


def _build(reps=1, ablate_dve=False, dve_quarter=False, pattern=None):
    nc = bacc.Bacc(None)
    Tanh = mybir.ActivationFunctionType.Tanh

    XTd = nc.declare_dram_parameter("XT", [DA, B], BF, isOutput=False)
    W0d = nc.declare_dram_parameter("W0a", [DA, N], BF, isOutput=False)
    Ud = {g: nc.declare_dram_parameter(f"U{g}a", [L, DA, N], BF, isOutput=False)
          for g in GATES}
    Wd = {g: nc.declare_dram_parameter(f"W{g}", [L, N, N], BF, isOutput=False)
          for g in GATES}
    Wfd = nc.declare_dram_parameter("Wf", [N, 1], BF, isOutput=False)
    bfd = nc.declare_dram_parameter("bfc", [1, 1], BF, isOutput=False)
    OUT = nc.declare_dram_parameter("out", [B, 1], FP, isOutput=True)

    with tile.TileContext(nc) as tc, ExitStack() as ctx:
        consts = ctx.enter_context(tc.tile_pool(name="consts", bufs=1))
        xtpool = ctx.enter_context(tc.tile_pool(name="xt", bufs=8))
        spool = ctx.enter_context(tc.tile_pool(name="s", bufs=8))
        zpool = ctx.enter_context(tc.tile_pool(name="z", bufs=4))
        gpool = ctx.enter_context(tc.tile_pool(name="g", bufs=4))
        rpool = ctx.enter_context(tc.tile_pool(name="r", bufs=4))
        hpool = ctx.enter_context(tc.tile_pool(name="h", bufs=4))
        opool = ctx.enter_context(tc.tile_pool(name="o", bufs=2))
        # pair-granular PSUM: each tile spans TWO banks ([128, 2, 512] fp32)
        psum = ctx.enter_context(tc.tile_pool(name="psum", bufs=3, space="PSUM"))
        psum_f = ctx.enter_context(tc.tile_pool(name="psum_f", bufs=2, space="PSUM"))

        # --- resident weights, natural (k-major) layout, via SWDGE ---
        def wdma(out, in_):
            nc.gpsimd.dma_start(out=out, in_=in_)

        w0 = consts.tile([DA, N], BF)
        bfc = consts.tile([1, 1], BF)
        u0, w0g, u12, w12 = {}, {}, {}, {}
        for g in GATES:
            u0[g] = consts.tile([DA, N], BF, name=f"u0_{g}")
            w0g[g] = consts.tile([128, KT, N], BF, name=f"w0_{g}")
            u12[g] = consts.tile([DA, L - 1, N], BF, name=f"u12_{g}")
            w12[g] = consts.tile([128, L - 1, KT, N], BF, name=f"w12_{g}")
        wf = consts.tile([128, KT], BF)

        def u_ap(g, l, c0, c1):
            return u0[g][:, c0:c1] if l == 0 else u12[g][:, l - 1, c0:c1]

        def w_ap(g, l, kt, c0, c1):
            return (w0g[g][:, kt, c0:c1] if l == 0
                    else w12[g][:, l - 1, kt, c0:c1])

        def emit_weight_dmas():
            nc.sync.dma_start(out=bfc[:], in_=bfd[:])
            wdma(w0[:], W0d[:])
            # first-consumed layer-0 weights first (gate order r,z,g,h)
            for g in ("r", "z", "g", "h"):
                wdma(u0[g][:], Ud[g][0].rearrange("p n -> p n"))
                wdma(w0g[g][:, 0:2],
                     Wd[g][0, 0:256].rearrange("(kt p) n -> p kt n", p=128))
                wdma(w0g[g][:, 2:4],
                     Wd[g][0, 256:512].rearrange("(kt p) n -> p kt n", p=128))
            for g in ("r", "z", "g", "h"):
                wdma(u12[g][:], Ud[g][1:3].rearrange("l p n -> p l n"))
                wdma(w12[g][:, 0],
                     Wd[g][1].rearrange("(kt p) n -> p kt n", p=128))
            for g in ("r", "z", "g", "h"):
                wdma(w12[g][:, 1],
                     Wd[g][2].rearrange("(kt p) n -> p kt n", p=128))
            wdma(wf[:], Wfd[:].rearrange("(kt p) o -> p (kt o)", p=128))

        sub, mult = mybir.AluOpType.subtract, mybir.AluOpType.mult

        def load_xt(c):
            xt = xtpool.tile([DA, BT], BF)
            if c == 0:
                h = BT // 2
                nc.sync.dma_start(out=xt[:, 0:h], in_=XTd[:, 0:h])
                nc.sync.dma_start(out=xt[:, h:BT], in_=XTd[:, h:BT])
            else:
                eng = nc.scalar if c == 1 else nc.sync
                eng.dma_start(out=xt[:], in_=XTd[:, c * BT:(c + 1) * BT])
            return xt

        def emit_s0_pair(xt, s, np_):
            # S0 = tanh(X_aug @ W0_aug), one two-bank pair group
            acc = psum.tile([128, 2, BT], FP, name="acc")
            for i in range(2):
                nt = 2 * np_ + i
                nc.tensor.matmul(acc[:, i, :], w0[:, nt * 128:(nt + 1) * 128],
                                 xt[:], start=True, stop=True)
            nc.scalar.activation(s[:, 2 * np_:2 * np_ + 2, :], acc[:], Tanh)

        def emit_xu(acc, g, l, xt, np_):
            # the two S-independent X@U matmuls of a pair group
            for i in range(2):
                nt = 2 * np_ + i
                nc.tensor.matmul(
                    acc[:, i, :], u_ap(g, l, nt * 128, (nt + 1) * 128),
                    xt[:], start=True, stop=False)

        def emit_sw(acc, g, l, src, np_, dest):
            # S@W contraction kt-MAJOR across both bank slices (first-half
            # S update unlocks 4 of 8 matmuls early), then the pair ACT
            for kt in range(KT):
                for i in range(2):
                    nt = 2 * np_ + i
                    nc.tensor.matmul(
                        acc[:, i, :],
                        w_ap(g, l, kt, nt * 128, (nt + 1) * 128),
                        src[:, kt, :], start=False, stop=(kt == KT - 1))
            nc.scalar.activation(dest[:, 2 * np_:2 * np_ + 2, :], acc[:], Tanh)

        def emit_gate_pair(g, l, xt, src, np_, dest):
            acc = psum.tile([128, 2, BT], FP, name="acc")
            emit_xu(acc, g, l, xt, np_)
            emit_sw(acc, g, l, src, np_, dest)

        def chunk_units(c, xt, s, bnd=()):
            """Generator: 31 work units (10 per layer + final).  Two of
            these run phase-offset-interleaved so every dependency stall
            of one chunk is covered by the other's matmul units.  `bnd`
            closures (S0 prefetch pair-groups) are emitted one per layer
            boundary, right after the S-update DVE batch: pure-PE work in
            the exact window where this chunk's next-layer matmuls wait
            on the update chain."""
            bnd = list(bnd)
            for l in range(L):
                rt = rpool.tile([128, NT, BT], BF)
                zt = zpool.tile([128, NT, BT], BF)
                gt = gpool.tile([128, NT, BT], BF)
                ht = hpool.tile([128, NT, BT], BF)
                # R first: hides the R-ACT -> R-mul -> H chain under Z/G
                for np_ in range(NP):
                    emit_gate_pair("r", l, xt, s, np_, rt)
                    yield
                for np_ in range(NP):
                    emit_gate_pair("z", l, xt, s, np_, zt)
                    yield
                # DVE 1: R <- S*R (feeds H); Z <- Z*S (in place, reads the
                # OLD S before the layer-end sub overwrites it)
                if not ablate_dve:
                    for hf in range(2):
                        cs = slice(2 * hf, 2 * hf + 2)
                        nc.vector.tensor_mul(rt[:, cs, :], s[:, cs, :],
                                             rt[:, cs, :])
                    for hf in range(2):
                        cs = slice(2 * hf, 2 * hf + 2)
                        nc.vector.tensor_mul(zt[:, cs, :], zt[:, cs, :],
                                             s[:, cs, :])
                yield
                for np_ in range(NP):
                    emit_gate_pair("g", l, xt, s, np_, gt)
                    yield
                for np_ in range(NP):
                    emit_gate_pair("h", l, xt, s if ablate_dve else rt,
                                   np_, ht)
                    yield
                # DVE 2: S = (Z*S) - (G-1)*H, half-gate granular.
                # The two halves are SEPARATE units: in the strict DVE
                # FIFO the h1 batch (waiting on H1-ACT) then sits behind
                # three foreign units' ready DVE ops instead of blocking
                # them head-of-line.
                if not ablate_dve:
                    cs = slice(0, 2)
                    nc.vector.scalar_tensor_tensor(
                        gt[:, cs, :], gt[:, cs, :], 1.0, ht[:, cs, :],
                        op0=sub, op1=mult)          # (G-1)*H
                    nc.vector.tensor_sub(s[:, cs, :], zt[:, cs, :],
                                         gt[:, cs, :])
                yield
                if not ablate_dve:
                    cs = slice(2, 4)
                    nc.vector.scalar_tensor_tensor(
                        gt[:, cs, :], gt[:, cs, :], 1.0, ht[:, cs, :],
                        op0=sub, op1=mult)          # (G-1)*H
                    nc.vector.tensor_sub(s[:, cs, :], zt[:, cs, :],
                                         gt[:, cs, :])
                if bnd:
                    bnd.pop(0)()
                yield
            emit_final(c, s, xt)
            yield

        def emit_final(c, s, xt_live):
            # out = S @ Wf + bf (bf lands via a K=1 matmul on the ones row)
            accf = psum_f.tile([1, BT], FP)
            nc.tensor.matmul(accf[:], bfc[:], xt_live[0:1, :],
                             start=True, stop=False)
            for kt in range(KT):
                nc.tensor.matmul(accf[:], wf[:, kt:kt + 1], s[:, kt, :],
                                 start=False, stop=(kt == KT - 1))
            ot = opool.tile([1, BT], FP)
            nc.scalar.activation(ot[:], accf[:],
                                 mybir.ActivationFunctionType.Copy)
            r0 = c * BT
            nc.sync.dma_start(out=OUT[r0:r0 + BT, 0:1].rearrange("b o -> o b"),
                              in_=ot[:])

        SENTINEL = object()
        W = 4                   # interleave width: chunks per quad

        def pull(g):
            return next(g, SENTINEL) is not SENTINEL

        def emit_all():
            # Four chunks run round-robin, one unit each: every chunk's
            # dependent next-layer matmuls enter the PE FIFO ~3 foreign
            # units (~6us) after their producer chain started, so the
            # ACT->DVE S-update latency is fully hidden.
            n_chunks = B // BT
            xts = {c: load_xt(c) for c in range(W)}
            svec = []
            for c in range(W):
                sc = spool.tile([128, KT, BT], BF, name="s")
                svec.append(sc)
                if c == 0:
                    # batch-halved chunk-0 S0: PE starts on the first
                    # xt half-transfer
                    for h in range(2):
                        c0, c1 = h * 256, (h + 1) * 256
                        for np_ in range(NP):
                            acc = psum.tile([128, 2, BT], FP, name="acc")
                            for i in range(2):
                                nt = 2 * np_ + i
                                nc.tensor.matmul(
                                    acc[:, i, 0:256],
                                    w0[:, nt * 128:(nt + 1) * 128],
                                    xts[0][:, c0:c1], start=True, stop=True)
                            nc.scalar.activation(
                                sc[:, 2 * np_:2 * np_ + 2, c0:c1],
                                acc[:, :, 0:256], Tanh)
                else:
                    for np_ in range(NP):
                        emit_s0_pair(xts[c], sc, np_)
            for q in range(n_chunks // W):
                c0 = q * W
                bnds = [() for _ in range(W)]
                nxt = []
                if c0 + W < n_chunks:
                    for i in range(W):
                        xts[c0 + W + i] = load_xt(c0 + W + i)
                    nxt = [spool.tile([128, KT, BT], BF, name="s")
                           for _ in range(W)]
                    # each chunk's l0/l1 boundary slots prefetch the S0 of
                    # its successor chunk in the next quad
                    bnds = [[(lambda i=i, np_=np_:
                              emit_s0_pair(xts[c0 + W + i], nxt[i], np_))
                             for np_ in range(NP)] for i in range(W)]
                gens = [chunk_units(c0 + i, xts[c0 + i], svec[i], bnds[i])
                        for i in range(W)]
                alive = [True] * W
                while any(alive):
                    for i in range(W):
                        if alive[i]:
                            alive[i] = pull(gens[i])
                if nxt:
                    svec = nxt

        emit_weight_dmas()
        if reps == 1:
            emit_all()
        else:           # device-side repetition loop, for benchmarking only
            with tc.For_i(0, reps):
                emit_all()

    nc.compile()
    return nc


_NC = None


def _get_nc():
    global _NC
    if _NC is None:
        _NC = _build()
    return _NC


def _bf(a):
    return np.ascontiguousarray(
        np.asarray(a, np.float32).astype(ml_dtypes.bfloat16))


def prep_shared(inputs):
    """bf16-convert weights; augment U-type weights with their bias as
    ROW 0 (matches the ones row at partition 0 of XT)."""
    shared = {}
    for g in GATES:
        shared[f"W{g}"] = _bf(inputs[f"W{g}"])
        U = np.asarray(inputs[f"U{g}"], np.float32)          # [L, D, N]
        b = np.asarray(inputs[f"b{g}"], np.float32)          # [L, 1, N]
        shared[f"U{g}a"] = _bf(
            np.concatenate([b.reshape(L, 1, N), U], axis=1))  # [L, DA, N]
    W0 = np.asarray(inputs["W0"], np.float32)                # [D, N]
    b0 = np.asarray(inputs["b0"], np.float32)                # [1, N]
    shared["W0a"] = _bf(np.concatenate([b0, W0], axis=0))
    shared["Wf"] = _bf(inputs["Wf"])
    shared["bfc"] = _bf(np.asarray(inputs["bf"], np.float32).reshape(1, 1))
    return shared


def prep_xt(Xcore):
    """[B, D] batch-major core shard -> [DA, B] bf16 feature-major,
    ones row 0."""
    Xt = np.asarray(Xcore, np.float32).T                     # [D, B]
    ones = np.ones((1, Xt.shape[1]), np.float32)
    return _bf(np.concatenate([ones, Xt], axis=0))


def _run(inputs, **kw):
    nc = _get_nc()
    shared = prep_shared(inputs)
    X = np.asarray(inputs["X"], np.float32)
    in_maps = [dict(shared, XT=prep_xt(X[i * B:(i + 1) * B]))
               for i in range(N_CORES)]
    res = run_bass_kernel_spmd(nc, in_maps, list(range(N_CORES)), **kw)
    out = np.concatenate([res.results[i]["out"] for i in range(N_CORES)], axis=0)
    return out, res


def kernel(**inputs) -> np.ndarray:
    out, _ = _run(inputs)
    return out
